# revision 10
# baseline (speedup 1.0000x reference)
"""Trainium2 Bass kernel for nn_DendriticCompartment (dense_mlp).

reference math:
    gates = sigmoid(x @ W_gate.T + b_gate)                      # (B, 4)
    seg_s = x @ W_seg[s].T + b_seg[s]                           # (B, 2048) per s
    plateau_s = sigmoid(5*(seg_s - thr_s))
    stacked_s = seg_s * plateau_s * gates[:, s:s+1]
    out = sum_s stacked_s + 0.1*sign(prod_s stacked_s)*prod_s|stacked_s|^(1/4)

Sharding: data-parallel on batch B (8192 -> 1024 cols per core); every core
computes all 4 segments and the full d_state for its batch slice.

Everything is computed TRANSPOSED on device: out^T[d, b] tiles with d on
partitions and batch on the free dim.  x is pre-transposed on the host so the
moving matmul operand (x^T tiles) loads with natural contiguous DMA and the
tensor engine never runs a single transpose.  W^T blocks (host pre-blocked)
are the stationary operands, streamed from HBM double-buffered.  Gates come
out of a 4-row matmul as gates^T[s, b] and are broadcast across 128
partitions with a tiny selector matmul.  The host transposes the per-core
[2048, 1024] results back when gathering.

Matmul inputs are bf16 (emulated end-to-end rel err 6e-3 vs the 2e-2 gate;
accumulation stays fp32 in PSUM and the whole epilogue is fp32).  bf16 avoids
the fp32r per-matmul stationary self-load (~53ns per 213ns matmul) - bf16
LDWEIGHTS are separate instructions the PE pulls ahead into the background
weight buffer, so the per-core tensor-engine time approaches the pure
streaming roofline: 8192*2048*1024 MACs / (128*128/cyc) / 2.4GHz = 437us.
"""
import numpy as np
import ml_dtypes
from contextlib import ExitStack

import concourse.bass as bass
import concourse.mybir as mybir
import concourse.tile as tile
from concourse import bacc
from concourse.bass_utils import run_bass_kernel_spmd

FP32 = mybir.dt.float32
BF16 = mybir.dt.bfloat16
AF = mybir.ActivationFunctionType
ALU = mybir.AluOpType
NPBF = ml_dtypes.bfloat16

B, D_IN, D_STATE, NSEG = 8192, 2048, 2048, 4
NCORES = 8
BSL = B // NCORES                  # 1024 batch columns per core
P = 128
KT = D_IN // P                     # 16 contraction tiles
NDH = D_STATE // P                 # 16 d-blocks of the output
NDB = NSEG * NDH                   # 64 weight blocks (dh-major, then s)
NQ = 4                             # x^T quarter tiles per rep
JQ = KT // NQ                      # 4 k-tiles per quarter
H = 512                            # psum bank width in fp32


def build_nc(repeats=1):
    nc = bacc.Bacc("TRN2", debug=False, target_bir_lowering=False,
                   num_devices=NCORES)

    xt_d = nc.dram_tensor("xt", [D_IN, BSL], BF16, kind="ExternalInput").ap()
    w_d = nc.dram_tensor("w", [NDB, P, KT, P], BF16, kind="ExternalInput").ap()
    wg_d = nc.dram_tensor("wg", [P, KT * NSEG], BF16, kind="ExternalInput").ap()
    bg_d = nc.dram_tensor("bg", [NSEG], FP32, kind="ExternalInput").ap()
    sel_d = nc.dram_tensor("sel", [NSEG, NSEG * P], BF16, kind="ExternalInput").ap()
    cb_d = nc.dram_tensor("cb", [P, 2 * NDB], FP32, kind="ExternalInput").ap()
    out_d = nc.dram_tensor("out", [D_STATE, BSL], FP32, kind="ExternalOutput").ap()

    with tile.TileContext(nc) as tc, ExitStack() as ctx:
        const = ctx.enter_context(tc.tile_pool(name="const", bufs=1))
        xt_p = ctx.enter_context(tc.tile_pool(name="xt", bufs=2))
        w_p = ctx.enter_context(tc.tile_pool(name="w", bufs=4))
        gbc_p = ctx.enter_context(tc.tile_pool(name="gbc", bufs=1))
        pl_p = ctx.enter_context(tc.tile_pool(name="pl", bufs=2))
        st_p = ctx.enter_context(tc.tile_pool(name="st", bufs=2))
        mx_p = ctx.enter_context(tc.tile_pool(name="mx", bufs=1))
        tl_p = ctx.enter_context(tc.tile_pool(name="tl", bufs=2))
        ps_m = ctx.enter_context(tc.tile_pool(name="ps_m", bufs=2, space="PSUM"))
        ps_g = ctx.enter_context(tc.tile_pool(name="ps_g", bufs=2, space="PSUM"))

        wgt = const.tile([P, KT, NSEG], BF16)
        nc.sync.dma_start(
            out=wgt[:], in_=wg_d.rearrange("p (kt s) -> p kt s", s=NSEG))
        selt = const.tile([NSEG, NSEG * P], BF16)
        nc.sync.dma_start(out=selt[:], in_=sel_d)
        bgp = const.tile([NSEG, 1], FP32)
        nc.gpsimd.dma_start(
            out=bgp[:],
            in_=bass.AP(tensor=bg_d.tensor, offset=bg_d.offset,
                        ap=[[1, NSEG], [1, 1]]))
        cbt = const.tile([P, 2 * NDB], FP32)
        nc.gpsimd.dma_start(out=cbt[:], in_=cb_d)

        xt_r = xt_d.rearrange("(q j p) b -> q p j b", p=P, j=JQ)
        out_r = out_d.rearrange("(dh p) b -> dh p b", p=P)

        for rep in range(repeats):
            # ---- load x^T slice, quarters on alternating queues ----
            xq = []
            for q in range(NQ):
                xqt = xt_p.tile([P, JQ, BSL], BF16, tag=f"xq{q}")
                eng = nc.sync if q % 2 == 0 else nc.gpsimd
                eng.dma_start(out=xqt[:], in_=xt_r[q])
                xq.append(xqt)

            # ---- gates: gacc[s, b] = sum_i Wg[s,i] xT[i,b] ----
            gacc = ps_g.tile([NSEG, BSL], FP32, tag="g")
            for kt in range(KT):
                q, j = divmod(kt, JQ)
                nc.tensor.matmul(gacc[:, 0:H], wgt[:, kt, :], xq[q][:, j, 0:H],
                                 start=(kt == 0), stop=(kt == KT - 1))
                nc.tensor.matmul(gacc[:, H:BSL], wgt[:, kt, :],
                                 xq[q][:, j, H:BSL],
                                 start=(kt == 0), stop=(kt == KT - 1))
            gsig = tl_p.tile([NSEG, BSL], BF16, tag="gsig", bufs=1)
            nc.scalar.activation(gsig[:], gacc[:], AF.Sigmoid, bias=bgp[:])
            # broadcast each segment's gate row to 128 partitions via matmul
            gbc = gbc_p.tile([P, NSEG, BSL], FP32, tag="gbc")
            for s in range(NSEG):
                gb = ps_g.tile([P, BSL], FP32, tag="g")
                lhs = selt[:, s * P:(s + 1) * P]
                nc.tensor.matmul(gb[:, 0:H], lhs, gsig[:, 0:H],
                                 start=True, stop=True)
                nc.tensor.matmul(gb[:, H:BSL], lhs, gsig[:, H:BSL],
                                 start=True, stop=True)
                nc.scalar.copy(out=gbc[:, s, :], in_=gb[:])

            # ---- main loop: 64 weight blocks, dh-major ----
            for dh in range(NDH):
                sts = {}
                for s in range(NSEG):
                    db = dh * NSEG + s
                    w = w_p.tile([P, KT, P], BF16, tag="w")
                    eng = nc.sync if db % 2 == 0 else nc.gpsimd
                    eng.dma_start(out=w[:], in_=w_d[db])
                    acc = ps_m.tile([P, BSL], FP32, tag="acc")
                    for kt in range(KT):
                        q, j = divmod(kt, JQ)
                        for c in (0, 2, 1, 3):
                            # N=256 matmuls beat N=512 by ~6%, and visiting
                            # the two PSUM banks alternately (0,2,1,3) saves
                            # another ~6% (bank write-port settle).
                            # start=True clears has_written for the WHOLE
                            # 2KB bank, so only the first 256-chunk of each
                            # bank may carry it, and it must come first.
                            nc.tensor.matmul(
                                acc[:, c * 256:(c + 1) * 256], w[:, kt, :],
                                xq[q][:, j, c * 256:(c + 1) * 256],
                                start=(kt == 0 and c % 2 == 0),
                                stop=(kt == KT - 1))
                    # plateau = sigmoid(5*seg - 5*thr) = sigmoid(5*acc + c1)
                    pl = pl_p.tile([P, BSL], FP32, tag="pl")
                    nc.scalar.activation(pl[:], acc[:], AF.Sigmoid, scale=5.0,
                                         bias=cbt[:, NDB + db:NDB + db + 1])
                    # st = (acc + b1) * plateau, then *= gate
                    st = st_p.tile([P, BSL], FP32, tag=f"st{s}")
                    nc.vector.scalar_tensor_tensor(
                        out=st[:], in0=acc[:], scalar=cbt[:, db:db + 1],
                        in1=pl[:], op0=ALU.add, op1=ALU.mult)
                    nc.gpsimd.tensor_mul(st[:], st[:], gbc[:, s, :])
                    sts[s] = st

                s01 = mx_p.tile([P, BSL], FP32, tag="s01")
                nc.gpsimd.tensor_add(s01[:], sts[0][:], sts[1][:])
                s23 = mx_p.tile([P, BSL], FP32, tag="s23")
                nc.gpsimd.tensor_add(s23[:], sts[2][:], sts[3][:])
                p01 = mx_p.tile([P, BSL], FP32, tag="p01")
                nc.gpsimd.tensor_mul(p01[:], sts[0][:], sts[1][:])
                p23 = mx_p.tile([P, BSL], FP32, tag="p23")
                nc.gpsimd.tensor_mul(p23[:], sts[2][:], sts[3][:])
                ssum = tl_p.tile([P, BSL], FP32, tag="ssum")
                nc.vector.tensor_add(ssum[:], s01[:], s23[:])
                pprod = tl_p.tile([P, BSL], FP32, tag="pprod", bufs=1)
                nc.vector.tensor_mul(pprod[:], p01[:], p23[:])
                sgn = tl_p.tile([P, BSL], FP32, tag="sgn")
                nc.scalar.sign(sgn[:], pprod[:])
                ab = tl_p.tile([P, BSL], FP32, tag="ab", bufs=1)
                nc.vector.tensor_mul(ab[:], pprod[:], sgn[:])
                nc.scalar.sqrt(ab[:], ab[:])
                nc.scalar.sqrt(ab[:], ab[:])
                res = tl_p.tile([P, BSL], FP32, tag="res")
                nc.vector.scalar_tensor_tensor(
                    out=res[:], in0=sgn[:], scalar=0.1, in1=ab[:],
                    op0=ALU.mult, op1=ALU.mult)
                nc.gpsimd.tensor_add(res[:], res[:], ssum[:])
                nc.sync.dma_start(out=out_r[dh], in_=res[:])

    nc.compile()
    return nc


_NC_CACHE = {}


def _get_nc():
    if "nc" not in _NC_CACHE:
        _NC_CACHE["nc"] = build_nc()
    return _NC_CACHE["nc"]


def make_in_maps(x, W_seg, b_seg, threshold, W_gate, b_gate):
    x = np.asarray(x, dtype=np.float32)
    xT = np.ascontiguousarray(x.T.astype(NPBF))                     # [2048, 8192]

    # W blocks: w[db=dh*4+s, ii, kt, dd] = W_seg[s, dh*128+dd, kt*128+ii]
    Wb = np.asarray(W_seg, dtype=np.float32).reshape(NSEG, NDH, P, KT, P)
    w_arr = np.ascontiguousarray(
        Wb.transpose(1, 0, 4, 3, 2).astype(NPBF)).reshape(NDB, P, KT, P)

    # wg[p, kt*4+s] = W_gate[s, kt*128+p]
    wg_arr = np.ascontiguousarray(
        np.asarray(W_gate, dtype=np.float32).T.reshape(KT, P, NSEG)
        .transpose(1, 0, 2).astype(NPBF)).reshape(P, KT * NSEG)

    sel = np.zeros((NSEG, NSEG * P), dtype=NPBF)
    for s in range(NSEG):
        sel[s, s * P:(s + 1) * P] = 1.0

    # cb[:, db] = b1 column; cb[:, NDB+db] = 5*(b1 - thr) column
    bs = np.asarray(b_seg, dtype=np.float32).reshape(NSEG, NDH, P)
    th = np.asarray(threshold, dtype=np.float32).reshape(NSEG, NDH, P)
    cb = np.empty((P, 2 * NDB), dtype=np.float32)
    cb[:, :NDB] = bs.transpose(2, 1, 0).reshape(P, NDB)
    cb[:, NDB:] = (5.0 * (bs - th)).transpose(2, 1, 0).reshape(P, NDB)

    bg = np.asarray(b_gate, dtype=np.float32)

    in_maps = []
    for c in range(NCORES):
        in_maps.append({
            "xt": np.ascontiguousarray(xT[:, c * BSL:(c + 1) * BSL]),
            "w": w_arr,
            "wg": wg_arr,
            "bg": bg,
            "sel": sel,
            "cb": cb,
        })
    return in_maps


def kernel(x, W_seg, b_seg, threshold, W_gate, b_gate):
    nc = _get_nc()
    in_maps = make_in_maps(x, W_seg, b_seg, threshold, W_gate, b_gate)
    res = run_bass_kernel_spmd(nc, in_maps, core_ids=list(range(NCORES)))
    return np.concatenate(
        [res.results[c]["out"].T for c in range(NCORES)], axis=0)
